# revision 68
# baseline (speedup 1.0000x reference)
"""Trainium2 Bass kernel for nn_Attentive_VLP_LSTM.

kernel(**inputs) takes the FULL unsharded inputs (numpy) and returns the
FULL [B, T, 3] output. Batch is sharded over 8 NeuronCores (32 rows
each); each core runs a fully-unrolled T=256 recurrent Bass/Tile program.

v2 design (latency-oriented — the kernel is one long serial dependency
chain per step; engines are mostly idle):
  - ACT table `exp_and_others` (exp, tanh, relu, square): LSTM gates use
    native tanh with sigmoid(x) = 0.5 + 0.5*tanh(x/2) (ifo weight
    columns pre-scaled 0.5 on host). No table switches.
  - The distance factor (d^2+eps)^-1/2 and the LayerNorm rstd are
    computed on DVE with the bit-trick inverse sqrt + 1 Newton step
    (max rel err ~1.8e-3), removing ACT Ln from the chain.
  - All matmuls run in bf16 (4x fewer PE cycles/row than fp32) except
    the distance matmul (fp32: |p-x|^2 by expansion cancels
    catastrophically in bf16). PSUM accumulation is fp32 throughout.
  - Attention softmax denominators are aggregated with group-sum rows
    REPLICATED to the full 96-row agg layout, so normalization is one
    reciprocal + one multiply (no 12->96 broadcast matmul, no copy).
  - LN stats are replicated x3 by the stationary ones matmul so the
    final rstd multiply needs no broadcast either.
  - Everything that doesn't depend on the current step's prediction
    (rss-part + h-part of the gate matmuls, dist matmuls, inverse-sqrt
    chain, next-step q-MLP K=1 matmul, DMA) is emitted so it runs in
    engine-idle windows off the critical chain.
  - The two branches out of pred -- [q-MLP -> relu -> score matmuls ->
    exp] and [prevaux -> dist matmul -> inverse-sqrt] -- are balanced
    to within ~60ns at the u*rsd join. Do NOT weaken the rsd Newton
    step: seed-only inverse sqrt (3.4% err) makes the closed-loop
    trajectory diverge (rel err ~1.3); 1 Newton (0.18%) holds 6e-3.
"""

import os
import sys

import numpy as np

for _p in ("/opt/trn_rl_repo", "/root/.axon_site", "/root/.axon_site/_ro/pypackages"):
    if _p not in sys.path and os.path.isdir(_p):
        sys.path.append(_p)

import ml_dtypes

import concourse.bass as bass
import concourse.tile as tile
from concourse import bacc, mybir
from concourse.bass_utils import run_bass_kernel_spmd

# Pin every activation to ONE table set so no LoadActFuncSet thrash:
# exp, tanh, relu, square all live in `exp_and_others`.
_KEEP_ACT_SET = "exp_and_others"
_orig_get_act_tables = bacc.get_activation_tables


def _patched_get_act_tables(arch):
    tabs = _orig_get_act_tables(arch)
    return {name: (fns if name == _KEEP_ACT_SET else set())
            for name, fns in tabs.items()}


bacc.get_activation_tables = _patched_get_act_tables

B, T, RSS, L, FEAT, H = 256, 256, 12, 256, 8, 128
NCORES = 8
BL = B // NCORES  # 32 batch rows per core
R = RSS * BL      # 384
AFT = mybir.ActivationFunctionType
ALU = mybir.AluOpType
F32 = mybir.dt.float32
BF16 = mybir.dt.bfloat16
I32 = mybir.dt.int32
DIST_EPS = 1e-8
LN_EPS = 1e-5
MAGIC = 0x5F3759DF

# gate column-block order in the [128, 4*BL] gate psum: f, i, g, o
# (torch weight-row order is i, f, g, o). f/i/o pre-scaled 0.5 for
# sigmoid(x) = 0.5 + 0.5*tanh(0.5 x); g unscaled for tanh(g). The
# order puts f first (earliest tanh chunk unblocks the cell update)
# and keeps i,g adjacent for one fused ACT.
_GATE_ROWS = [(H, 2 * H), (0, H), (2 * H, 3 * H), (3 * H, 4 * H)]
_GATE_SCALE = [0.5, 0.5, 1.0, 0.5]


def _f32(x):
    return np.ascontiguousarray(np.asarray(x, np.float32))


def _bf(x):
    return np.ascontiguousarray(np.asarray(x, np.float32).astype(ml_dtypes.bfloat16))


def _host_prep(inputs):
    """Static marshalling: LED sort + 32-aligned slot layout, one-time
    MLPs, weight folds and bf16 casts."""
    gpf = _f32(inputs["global_led_pos_freq"])  # [L, 4]
    freq = gpf[:, 3]
    perm = np.argsort(freq, kind="stable")
    gpf_p = gpf[perm]

    relu = lambda x: np.maximum(x, np.float32(0))
    lin = lambda x, W, bb: x @ _f32(W).T + _f32(bb)

    led_feat = relu(lin(relu(lin(gpf_p, inputs["enc_W1"], inputs["enc_b1"])),
                        inputs["enc_W2"], inputs["enc_b2"]))  # [L, 8]
    keys = lin(relu(lin(np.concatenate([led_feat, gpf_p[:, :3]], 1),
                        inputs["k_W1"], inputs["k_b1"])),
               inputs["k_W2"], inputs["k_b2"])  # [L, 64]

    # --- padded slot layout: each freq group starts at a 32-boundary ---
    freq_p = gpf_p[:, 3]
    bounds = np.searchsorted(freq_p, np.arange(1, RSS + 2) - 0.5)
    slot_of_group = []   # list of (slot_base, r)
    src_list = []
    base = 0
    for r in range(RSS):
        g0, g1 = int(bounds[r]), int(bounds[r + 1])
        n = g1 - g0
        nslots = max(1, (n + 31) // 32)
        for s in range(nslots):
            slot_of_group.append((base + 32 * s, r))
        sl = -np.ones(nslots * 32, np.int64)
        sl[:n] = np.arange(g0, g1)
        src_list.append(sl)
        base += nslots * 32
    pad_src = np.concatenate(src_list)
    LP = ((base + 127) // 128) * 128
    pad_src = np.concatenate([pad_src, -np.ones(LP - base, np.int64)])
    NC = LP // 128
    real = pad_src >= 0
    # dummy slots so EVERY spsum lane is written each step (exp reads the
    # full [128, NC*BL] psum; unwritten lanes hold stale garbage -> inf
    # -> 0*inf = NaN in the aggregation matmul). KWT_e cols there are 0.
    # Prepended AND fed a constant zero moving operand so they carry no
    # relu dependency and run at step start, ahead of the real slots.
    ndummy = (LP - base) // 32
    slot_of_group = [(gb, 0) for gb in range(base, LP, 32)] + slot_of_group

    def expand(arr_p, fill=0.0):
        out = np.full((LP,) + arr_p.shape[1:], fill, np.float32)
        out[real] = arr_p[pad_src[real]]
        return out

    led_feat_e = expand(led_feat)
    pos_e = expand(gpf_p[:, :3], fill=1000.0)  # dummies far away
    keys_e = expand(keys)
    r_of_e = np.zeros(LP, np.int64)
    r_of_e[real] = np.rint(freq_p[pad_src[real]] - 1.0).astype(np.int64)

    KW = 2.0 * keys_e @ _f32(inputs["q_W2"])      # [LP, 64]
    kb2 = 2.0 * keys_e @ _f32(inputs["q_b2"])     # [LP]
    KWT_e = np.concatenate([KW.T, kb2[None, :]], 0)  # [65, LP]

    # prevaux rows: 0:3 pred, 3 ones, 4:32 pad, 32:35 pred^2 (32-aligned)
    # DIST_EPS folded into the |x|^2 row so dps = d^2 + eps directly.
    poshT = np.zeros((35, LP), np.float32)
    poshT[0:3] = -2.0 * pos_e.T
    poshT[3] = (pos_e * pos_e).sum(1) + DIST_EPS
    poshT[32:35] = 1.0

    # XWn: led_feat rows; XWd: group-sum rows replicated to the same
    # 96-row layout (row r*8+j gets its group's den for every j)
    XWn = np.zeros((NC, 128, 96), np.float32)
    XWd = np.zeros((NC, 128, 96), np.float32)
    for lp in range(LP):
        if not real[lp]:
            continue
        c, lr = divmod(lp, 128)
        r = r_of_e[lp]
        XWn[c, lr, r * 8:(r + 1) * 8] = led_feat_e[lp]
        XWd[c, lr, r * 8:(r + 1) * 8] = 1.0

    def gate_cols(Wt):  # [K, 512] torch-order cols -> [i|f|o|g], scaled
        blocks = [np.float32(s) * Wt[:, a:b]
                  for (a, b), s in zip(_GATE_ROWS, _GATE_SCALE)]
        return np.concatenate(blocks, 1)

    def wih_feature_rows(Wih):  # [512, 108] -> feature-major [108, 512]
        Wt = _f32(Wih).T
        out = np.zeros((108, 4 * H), np.float32)
        for r in range(RSS):
            out[96 + r] = Wt[r * 9]
            for j in range(FEAT):
                out[r * 8 + j] = Wt[r * 9 + 1 + j]
        return out

    bsum0 = _f32(inputs["bih0"]) + _f32(inputs["bhh0"])
    bsum1 = _f32(inputs["bih1"]) + _f32(inputs["bhh1"])
    A0 = wih_feature_rows(inputs["Wih0"])  # [108, 512]

    # layer-1 bias fallback: ACT tanh bias, pre-scaled per gate
    eb1 = np.zeros((H, 4), np.float32)
    for gi, ((a, b_), s) in enumerate(zip(_GATE_ROWS, _GATE_SCALE)):
        eb1[:, gi] = np.float32(s) * bsum1[a:b_]

    W1g = _f32(inputs["fc_W1"]) * _f32(inputs["ln_g"])[None, :]  # [64, H]
    b1f = (_f32(inputs["fc_W1"]) @ _f32(inputs["ln_b"])
           + _f32(inputs["fc_b1"]))                               # [64]
    W2T = np.concatenate([_f32(inputs["fc_W2"]).T,
                          np.zeros((1, 3), np.float32)], 0)       # [65, 3]

    consts = {
        "qW1a": _bf(_f32(inputs["q_W1"]).T[0:1]),          # [1, 64]
        "qW1b": _bf(_f32(inputs["q_W1"]).T[1:4]),          # [3, 64]
        "b1q": _f32(inputs["q_b1"])[:, None],              # [64, 1]
        "KWT_e": _bf(KWT_e),                               # [65, LP]
        "poshT": _f32(poshT),                              # [35, LP]
        # h states are kept DOUBLED (H = 2h, Z = 2c) so the sigmoid
        # (1+tanh)/2 halves fold into products; every weight that
        # consumes an h state absorbs the 0.5 here.
        "Wrss0T": _bf(gate_cols(np.concatenate(
            [A0[96:108], bsum0[None, :]], 0))),            # [13, 512]
        "Wagg0T": _bf(gate_cols(A0[0:96])),                # [96, 512]
        "Whh0T": _bf(0.5 * gate_cols(_f32(inputs["Whh0"]).T)),
        "Wih1T": _bf(0.5 * gate_cols(_f32(inputs["Wih1"]).T)),
        "Whh1T": _bf(0.5 * gate_cols(_f32(inputs["Whh1"]).T)),
        "eb1": eb1,                                        # [128, 4]
        "W1gT": _bf(0.5 * W1g.T),                          # [128, 64]
        "w1s": _bf(W1g.T.sum(0, keepdims=True)),           # [1, 64]
        "b1f": b1f[:, None],                               # [64, 1]
        "W2T": _bf(W2T),                                   # [65, 3]
        "b2": _f32(inputs["fc_b2"])[:, None],              # [3, 1]
        "ones3": _bf(np.full((128, 3), 1.0 / H)),          # [128, 3]
        "ones13": _bf(np.ones((1, 3))),                    # [1, 3]
        "zro65": _bf(np.zeros((65, BL))),                  # [65, BL]
        "epsrow": _bf(np.concatenate(
            [np.zeros((1, BL)), np.full((1, BL), LN_EPS)], 1)),  # [1, 2BL]
    }
    for c in range(NC):
        consts[f"XWn{c}"] = _bf(XWn[c])
        consts[f"XWd{c}"] = _bf(XWd[c])

    pred0 = gpf_p[:, :3].mean(0).astype(np.float32)
    init = {
        "prevb0": _bf(np.broadcast_to(pred0[:, None], (3, BL))),
        "prevaux0": np.concatenate(
            [np.broadcast_to(pred0[:, None], (3, BL)),
             np.ones((1, BL), np.float32),
             np.zeros((28, BL), np.float32),
             np.broadcast_to((pred0 * pred0)[:, None], (3, BL))],
            0).astype(np.float32),                          # [35, BL]
        "h00": _bf(np.zeros((H, BL))),
        "h1w0": _bf(np.zeros((H, 2 * BL))),
        "c00": np.zeros((H, BL), np.float32),
        "c10": np.zeros((H, BL), np.float32),
        "q1re0": _bf(np.concatenate([np.zeros((64, R)), np.ones((1, R))], 0)),
        "Are0": _bf(np.concatenate([np.zeros((64, BL)), np.ones((1, BL))], 0)),
    }
    meta = {
        "bias1_zero": bool(not np.any(bsum1)),
        "b2_zero": bool(not np.any(_f32(inputs["fc_b2"]))),
        "slots": slot_of_group,
        "ndummy": ndummy,
        "NC": NC,
        "LP": LP,
    }
    return consts, init, meta


def _per_core_rss(rss_core):
    """rss_core [BL, T, RSS] -> rss_q [T, R] bf16 and rssOT [13, T*BL]
    bf16 (12 rss rows + ones row for the layer-0 bias fold)."""
    rss_q = np.ascontiguousarray(
        rss_core.transpose(1, 2, 0).reshape(T, R))       # [t, r*BL+b]
    rssT = rss_core.transpose(2, 1, 0).reshape(RSS, T * BL)
    rssOT = np.concatenate([rssT, np.ones((1, T * BL), np.float32)], 0)
    return _bf(rss_q), _bf(rssOT)


def build_nc(consts, init, meta, nsteps=T, taps=()):
    nc = bacc.Bacc("TRN2", target_bir_lowering=False, debug=False,
                   num_devices=NCORES)
    NC = meta["NC"]
    slots = meta["slots"]
    tap_tiles = {}

    dram = {}
    for k, v in {**consts, **init}.items():
        dt = BF16 if v.dtype == ml_dtypes.bfloat16 else F32
        dram[k] = nc.dram_tensor(k, list(v.shape), dt,
                                 kind="ExternalInput").ap()
    dram["rss_q"] = nc.dram_tensor("rss_q", [T, R], BF16,
                                   kind="ExternalInput").ap()
    dram["rssOT"] = nc.dram_tensor("rssOT", [RSS + 1, T * BL], BF16,
                                   kind="ExternalInput").ap()
    d_out = nc.dram_tensor("out", [3, nsteps * BL], F32,
                           kind="ExternalOutput").ap()

    with tile.TileContext(nc) as tc:
        with (
            tc.tile_pool(name="const", bufs=1) as cpool,
            tc.tile_pool(name="state", bufs=1) as spool,
            tc.tile_pool(name="work", bufs=2) as wpool,
            tc.tile_pool(name="qrow", bufs=3) as qpool,
            # PSUM: 8 banks
            tc.tile_pool(name="pq1", bufs=1, space="PSUM") as pq1,
            tc.tile_pool(name="psc", bufs=1, space="PSUM") as psc,
            tc.tile_pool(name="pdp", bufs=1, space="PSUM") as pdp,
            tc.tile_pool(name="pP", bufs=1, space="PSUM") as pP,
            tc.tile_pool(name="pg0", bufs=1, space="PSUM") as pg0,
            tc.tile_pool(name="pg1", bufs=1, space="PSUM") as pg1,
            tc.tile_pool(name="psm", bufs=2, space="PSUM") as psm,
        ):
            cs = {}
            for k, v in consts.items():
                dt = BF16 if v.dtype == ml_dtypes.bfloat16 else F32
                t_ = cpool.tile(list(v.shape), dt, tag=k, name=k)
                nc.sync.dma_start(t_[:], dram[k][:])
                cs[k] = t_
            t_rssOT = cpool.tile([RSS + 1, T * BL], BF16, tag="rssOT",
                                 name="t_rssOT")
            nc.sync.dma_start(t_rssOT[:], dram["rssOT"][:])

            st = {}
            for k, shape, dt in [("prevb", [3, BL], BF16),
                                 ("prevaux", [35, BL], F32),
                                 ("h0", [H, BL], BF16),
                                 ("h1w", [H, 2 * BL], BF16),
                                 ("c0", [H, BL], F32),
                                 ("c1", [H, BL], F32),
                                 ("q1re", [65, R], BF16),
                                 ("Are", [65, BL], BF16),
                                 ("xTa", [96, BL], BF16)]:
                st[k] = spool.tile(shape, dt, tag=k, name="st_" + k)
            for k, src in [("prevb", "prevb0"), ("prevaux", "prevaux0"),
                           ("h0", "h00"), ("h1w", "h1w0"),
                           ("c0", "c00"), ("c1", "c10"),
                           ("q1re", "q1re0"), ("Are", "Are0")]:
                nc.sync.dma_start(st[k][:], dram[src][:])
            t_out = spool.tile([3, nsteps * BL], F32, tag="out_sb",
                               name="t_out")

            mm = nc.tensor.matmul
            act = nc.scalar.activation
            V = nc.vector
            G = nc.gpsimd

            def bc_r(ap3, ngroups=RSS):
                """[3, BL] AP -> broadcast [3, ngroups*BL]."""
                return bass.AP(ap3.tensor, ap3.offset,
                               [ap3.ap[0], [0, ngroups], ap3.ap[-1]])

            def invsqrt(x_ap, out_tile, tag, shape):
                """out = x^-1/2 via bit trick + 1 fused Newton step
                (5 serial DVE ops). x_ap fp32 (SBUF or PSUM)."""
                ti_ = wpool.tile(shape, I32, tag=tag + "i", name=tag + "i")
                V.tensor_scalar(ti_[:], x_ap.bitcast(I32), 1, None,
                                op0=ALU.logical_shift_right)
                ty = wpool.tile(shape, I32, tag=tag + "y", name=tag + "y")
                V.tensor_scalar(ty[:], ti_[:], MAGIC, -1,
                                op0=ALU.subtract, op1=ALU.mult)
                y0 = ty[:].bitcast(F32)
                ta = wpool.tile(shape, F32, tag=tag + "a", name=tag + "a")
                V.tensor_tensor(ta[:], y0, y0, op=ALU.mult)
                tb = wpool.tile(shape, F32, tag=tag + "b", name=tag + "b")
                V.scalar_tensor_tensor(tb[:], ta[:], -0.5, x_ap,
                                       op0=ALU.mult, op1=ALU.mult)
                V.scalar_tensor_tensor(out_tile[:], tb[:], 1.5, y0,
                                       op0=ALU.add, op1=ALU.mult)

            # step-0 prefetches: qrow + q1a matmul
            qr_next = qpool.tile([1, R], BF16, tag="qrow", name="qrow")
            nc.sync.dma_start(qr_next[:], dram["rss_q"][0:1, :])
            q1ps_next = pq1.tile([64, R], F32, tag="q1", name="q1ps")
            mm(q1ps_next[:], cs["qW1a"][:], qr_next[:],
               start=True, stop=False)

            for t in range(nsteps):
                q1ps = q1ps_next
                qrow = qr_next

                # dist matmul + inverse-sqrt factor (off-chain)
                dps = pdp.tile([128, NC * BL], F32, tag="ds", name="dps")
                for c in range(NC):
                    mm(dps[:, c * BL:(c + 1) * BL],
                       cs["poshT"][:, c * 128:(c + 1) * 128],
                       st["prevaux"][:], start=True, stop=True)
                rsd = wpool.tile([128, NC * BL], BF16, tag="rsd", name="rsd")
                invsqrt(dps[:], rsd, "dv", [128, NC * BL])
                # ---------- chain head: q-MLP prev part ----------
                mm(q1ps[:], cs["qW1b"][:], bc_r(st["prevb"][:]),
                   start=False, stop=True, skip_group_check=True)

                # off-chain gate matmuls (inputs ready from step t-1)
                # NOTE: PSUM start=True clears has_written for the whole
                # bank, so only the FIRST matmul into each gate bank per
                # step may carry it; later region-initializing writes rely
                # on has_written-driven overwrite (start=False).
                gps0 = pg0.tile([128, 4 * BL], F32, tag="g0", name="gps0")
                for gi in range(4):
                    mm(gps0[:, gi * BL:(gi + 1) * BL],
                       cs["Wrss0T"][:, gi * H:(gi + 1) * H],
                       t_rssOT[:, t * BL:(t + 1) * BL],
                       start=(gi == 0), stop=False, skip_group_check=True)
                for gi in range(4):
                    mm(gps0[:, gi * BL:(gi + 1) * BL],
                       cs["Whh0T"][:, gi * H:(gi + 1) * H], st["h0"][:],
                       start=False, stop=False, skip_group_check=True)
                gps1 = pg1.tile([128, 4 * BL], F32, tag="g1", name="gps1")
                for gi in range(4):
                    mm(gps1[:, gi * BL:(gi + 1) * BL],
                       cs["Whh1T"][:, gi * H:(gi + 1) * H],
                       st["h1w"][:, 0:BL], start=(gi == 0), stop=False,
                       skip_group_check=True)


                # ---------- chain: relu -> scores -> exp -> P ----------
                act(st["q1re"][0:64, :], q1ps[:], AFT.Relu,
                    bias=cs["b1q"][:, 0:1])

                spsum = psc.tile([128, NC * BL], F32, tag="sc", name="spsum")
                for si, (gb, r) in enumerate(slots):
                    c, lb = divmod(gb, 128)
                    mov = (cs["zro65"][:] if si < meta["ndummy"] else
                           st["q1re"][:, r * BL:(r + 1) * BL])
                    mm(spsum[lb:lb + 32, c * BL:(c + 1) * BL],
                       cs["KWT_e"][:, gb:gb + 32], mov,
                       start=True, stop=True, tile_position=(0, lb))

                # prefetches for t+1 fill the PE gap here
                if t + 1 < nsteps:
                    qr_next = qpool.tile([1, R], BF16, tag="qrow",
                                         name="qrow")
                    nc.sync.dma_start(qr_next[:],
                                      dram["rss_q"][t + 1:t + 2, :])
                    q1ps_next = pq1.tile([64, R], F32, tag="q1",
                                         name="q1ps")
                    mm(q1ps_next[:], cs["qW1a"][:], qr_next[:],
                       start=True, stop=False)

                u = wpool.tile([128, NC * BL], BF16, tag="u", name="u_sb")
                act(u[:], spsum[:], AFT.Exp, scale=0.5)
                if taps and t == nsteps - 1:
                    spdbg = wpool.tile([128, NC * BL], F32, tag="spd",
                                       name="spdbg")
                    V.tensor_copy(spdbg[:], spsum[:])
                    tap_tiles["spsum"] = spdbg
                    tap_tiles["u"] = u
                u2 = wpool.tile([128, NC * BL], BF16, tag="u2", name="u2")
                V.tensor_tensor(u2[:], u[:], rsd[:], op=ALU.mult)

                P2 = pP.tile([96, 2 * BL], F32, tag="P", name="P2ps")
                for c in range(NC):
                    mm(P2[:, BL:2 * BL], cs[f"XWd{c}"][:],
                       u2[:, c * BL:(c + 1) * BL],
                       start=(c == 0), stop=(c == NC - 1))
                for c in range(NC):
                    mm(P2[:, 0:BL], cs[f"XWn{c}"][:],
                       u2[:, c * BL:(c + 1) * BL],
                       start=(c == 0), stop=(c == NC - 1))
                rden = wpool.tile([96, BL], F32, tag="rden", name="rden")
                V.reciprocal(rden[:], P2[:, BL:2 * BL])
                V.tensor_tensor(st["xTa"][:], P2[:, 0:BL], rden[:],
                                op=ALU.mult)

                # ---------- two LSTM layers ----------
                for ly in range(2):
                    gps = gps0 if ly == 0 else gps1
                    cst = st["c0"] if ly == 0 else st["c1"]
                    if ly == 0:
                        for gi in range(4):
                            mm(gps[:, gi * BL:(gi + 1) * BL],
                               cs["Wagg0T"][:, gi * H:(gi + 1) * H],
                               st["xTa"][:], start=False, stop=True,
                               skip_group_check=True)
                    else:
                        for gi in range(4):
                            mm(gps[:, gi * BL:(gi + 1) * BL],
                               cs["Wih1T"][:, gi * H:(gi + 1) * H],
                               st["h0"][:], start=False, stop=True,
                               skip_group_check=True)

                    if taps and t == nsteps - 1:
                        gdbg = wpool.tile([128, 4 * BL], F32,
                                          tag=f"gd{ly}", name="gdbg")
                        V.tensor_copy(gdbg[:], gps[:])
                        tap_tiles[f"gps{ly}"] = gdbg
                    # cell kept doubled (Z = 2c):
                    #   t1 = (1+tanh(f/2))*Z   = 2*sig(f)*Z
                    #   t2 = (1+tanh(i/2))*tg  = 2*sig(i)*tg
                    #   Z' = 0.5*t1 + t2       = 2*c'
                    #   thc = tanh(0.5*Z')     = tanh(c')
                    #   H' = (1+tanh(o/2))*thc = 2*h'
                    th4 = wpool.tile([128, 4 * BL], BF16, tag=f"th{ly}",
                                     name="th4")
                    if ly == 0 or meta["bias1_zero"]:
                        act(th4[:], gps[:], AFT.Tanh)
                    else:
                        for gi in range(4):
                            act(th4[:, gi * BL:(gi + 1) * BL],
                                gps[:, gi * BL:(gi + 1) * BL], AFT.Tanh,
                                bias=cs["eb1"][:, gi:gi + 1])
                    t1 = wpool.tile([128, BL], F32, tag=f"p{ly}", name="t1")
                    V.scalar_tensor_tensor(t1[:], th4[:, 0:BL], 1.0,
                                           cst[:], op0=ALU.add, op1=ALU.mult)
                    t2 = wpool.tile([128, BL], F32, tag=f"q{ly}", name="t2")
                    V.scalar_tensor_tensor(t2[:], th4[:, BL:2 * BL], 1.0,
                                           th4[:, 2 * BL:3 * BL],
                                           op0=ALU.add, op1=ALU.mult)
                    V.scalar_tensor_tensor(cst[:], t1[:], 0.5, t2[:],
                                           op0=ALU.mult, op1=ALU.add)
                    thc = wpool.tile([128, BL], BF16, tag=f"tc{ly}",
                                     name="thct")
                    act(thc[:], cst[:], AFT.Tanh, scale=0.5)
                    hout = st["h0"][:] if ly == 0 else st["h1w"][:, 0:BL]
                    V.scalar_tensor_tensor(hout, th4[:, 3 * BL:4 * BL], 1.0,
                                           thc[:], op0=ALU.add, op1=ALU.mult)
                    if taps and t == nsteps - 1:
                        tap_tiles[f"th4{ly}"] = th4
                        tap_tiles[f"thc{ly}"] = thc

                # ---------- LayerNorm + fc head ----------
                # h1w col0 holds H1 = 2*h1; hsq column holds true h1^2
                V.scalar_tensor_tensor(st["h1w"][:, BL:2 * BL],
                                       st["h1w"][:, 0:BL], 0.25,
                                       st["h1w"][:, 0:BL],
                                       op0=ALU.mult, op1=ALU.mult)
                smallps = psm.tile([64, 4 * BL], F32, tag="sm", name="smps")
                stat = smallps[0:3, 0:2 * BL]
                statmu = smallps[0:3, 0:BL]
                statsq = smallps[0:3, BL:2 * BL]
                a2ps = smallps[0:64, 2 * BL:3 * BL]
                prps = smallps[0:3, 3 * BL:4 * BL]
                mm(stat, cs["ones13"][:], cs["epsrow"][:],
                   start=True, stop=False)  # eps row first (const inputs)
                mm(stat, cs["ones3"][:], st["h1w"][:],
                   start=False, stop=True)  # [mu | E[h^2]+eps] x3 rows

                negmu = wpool.tile([3, BL], BF16, tag="nmu", name="negmu")
                V.tensor_scalar(negmu[:], statmu, -0.5, None,
                                op0=ALU.mult)  # statmu = 2*mu
                # rstd branch FIRST: its vv reads the packed smallps psum
                # tile, and tile-granular dep tracking would otherwise
                # stall it behind the prps write into another region.
                m2 = wpool.tile([3, BL], F32, tag="m2", name="m2")
                V.tensor_tensor(m2[:], negmu[:], negmu[:], op=ALU.mult)
                vv = wpool.tile([3, BL], F32, tag="vv", name="vv")
                V.scalar_tensor_tensor(vv[:], statsq, 1.0, m2[:],
                                       op0=ALU.mult, op1=ALU.subtract)
                r3 = wpool.tile([3, BL], F32, tag="r3", name="r3")
                invsqrt(vv[:], r3, "lv", [3, BL])
                # fc branch (parallel with rstd branch)
                mm(a2ps, cs["W1gT"][:], st["h1w"][:, 0:BL],
                   start=True, stop=False)
                mm(a2ps, cs["w1s"][:], negmu[0:1, :],
                   start=False, stop=True)
                act(st["Are"][0:64, :], a2ps, AFT.Relu,
                    bias=cs["b1f"][:, 0:1])
                mm(prps, cs["W2T"][:], st["Are"][:], start=True, stop=True)

                # chain tail. The dist branch (prevaux -> dps -> invsqrt)
                # is the DEEPEST consumer of pred, so prevaux is written
                # first; t_out is an off-chain copy of it.
                osl = t_out[:, t * BL:(t + 1) * BL]
                if t + 1 < nsteps:
                    pa03 = st["prevaux"][0:3, :]
                    V.tensor_tensor(pa03, prps, r3[:], op=ALU.mult)
                    if not meta["b2_zero"]:
                        V.tensor_scalar(pa03, pa03, cs["b2"][:, 0:1], None,
                                        op0=ALU.add)
                    V.tensor_tensor(st["prevaux"][32:35, :], pa03, pa03,
                                    op=ALU.mult)
                    V.tensor_copy(st["prevb"][:], pa03)
                    V.tensor_copy(osl, pa03)
                else:
                    V.tensor_tensor(osl, prps, r3[:], op=ALU.mult)
                    if not meta["b2_zero"]:
                        V.tensor_scalar(osl, osl, cs["b2"][:, 0:1], None,
                                        op0=ALU.add)

                if taps and t == nsteps - 1:
                    tap_tiles.update(dict(
                        q1re=st["q1re"], xTa=st["xTa"], h0=st["h0"],
                        h1w=st["h1w"], Are=st["Are"], u2=u2, rsd=rsd,
                        r3=r3, negmu=negmu, vv=vv))

            nc.sync.dma_start(d_out[:], t_out[:])
            for name in taps:
                tt = tap_tiles[name]
                dt = tt[:].dtype
                shape = [tt[:].partition_size(), tt[:].free_size()]
                dto = nc.dram_tensor("tap_" + name, shape, dt,
                                     kind="ExternalOutput").ap()
                nc.sync.dma_start(dto[:], tt[:])

    nc.compile()
    return nc


def make_in_maps(consts, init, rss_seq):
    base = dict(consts)
    base.update(init)
    in_maps = []
    for k in range(NCORES):
        rss_q, rssOT = _per_core_rss(rss_seq[k * BL:(k + 1) * BL])
        m = dict(base)
        m["rss_q"] = rss_q
        m["rssOT"] = rssOT
        in_maps.append(m)
    return in_maps


def kernel(**inputs):
    rss_seq = _f32(inputs["rss_seq"])
    consts, init, meta = _host_prep(inputs)
    nc = build_nc(consts, init, meta, nsteps=T)
    in_maps = make_in_maps(consts, init, rss_seq)
    res = run_bass_kernel_spmd(nc, in_maps, core_ids=list(range(NCORES)))
    outs = []
    for k in range(NCORES):
        o = res.results[k]["out"]
        outs.append(np.asarray(o, np.float32).reshape(3, T, BL)
                    .transpose(2, 1, 0))
    return np.ascontiguousarray(np.concatenate(outs, 0))
